# revision 2
# baseline (speedup 1.0000x reference)
"""Trainium2 Bass kernel for nn_DynamicQuantizedLinear.

Computes out = x @ dequant(W).T + bias + residual where
  x:[64,4096] f32, W_q:[11008,4096] int8, scale:[11008,32] f16 (group size 128),
  bias/residual:[11008] f16.

Strategy (column-parallel over out_features, 8 cores):
  - Host: dequantize W to fp16 (exact: int8 * fp16-scale product rounded once),
    transpose to [in, out] so the contraction dim lands on SBUF partitions,
    shard out_features 1376 per core, replicate x as fp16 in [128, g, b] layout.
  - Device: for each of 32 K-groups, one [128,64] fp16 x-tile is the matmul
    stationary operand and the weight slab [128, 1376] streams as the moving
    operand into 3 PSUM banks (N=512/512/352), accumulating over groups.
    bias+residual enters PSUM via a K=1 ones-row matmul before the group loop.
  - Output [64, 1376] f32 per core, host concatenates along features.
"""

import numpy as np

OUT, IN, GS = 11008, 4096, 128
NG = IN // GS          # 32 groups
B = 64                 # batch rows
NCORES = 8
OPC = OUT // NCORES    # 1376 out features per core
CHUNKS = [(0, 512), (512, 512), (1024, OPC - 1024)]  # psum bank chunks

_NC_CACHE = None


def _build():
    global _NC_CACHE
    if _NC_CACHE is not None:
        return _NC_CACHE

    import concourse.bacc as bacc
    import concourse.tile as tile
    import concourse.bass as bass
    import concourse.mybir as mybir

    f16 = mybir.dt.float16
    f32 = mybir.dt.float32

    nc = bacc.Bacc(
        "TRN2", target_bir_lowering=False, debug=False, enable_asserts=False
    )
    wt = nc.dram_tensor("wt", [IN, OPC], f16, kind="ExternalInput").ap()
    xg = nc.dram_tensor("xg", [128, NG * B], f16, kind="ExternalInput").ap()
    br = nc.dram_tensor("br", [1, OPC], f16, kind="ExternalInput").ap()
    out = nc.dram_tensor("out", [B, OPC], f32, kind="ExternalOutput").ap()

    with tile.TileContext(nc) as tc:
        with (
            tc.tile_pool(name="xp", bufs=1) as xpool,
            tc.tile_pool(name="wp", bufs=NG) as wpool,
            tc.tile_pool(name="cp", bufs=1) as cpool,
            tc.tile_pool(name="op", bufs=1) as opool,
            tc.tile_pool(name="pp", bufs=1, space=bass.MemorySpace.PSUM) as pspool,
        ):
            xt = xpool.tile([128, NG * B], f16)
            nc.sync.dma_start(xt[:], xg[:])
            brt = cpool.tile([1, OPC], f16, tag="brt")
            nc.sync.dma_start(brt[:], br[:])
            ones = cpool.tile([1, B], f16, tag="ones")
            nc.vector.memset(ones[:], 1.0)

            ps = [
                pspool.tile([B, n], f32, tag=f"ps{i}", name=f"ps{i}")
                for i, (_, n) in enumerate(CHUNKS)
            ]
            # bias+residual: psum[b, o] = sum_{k=1} ones[k, b] * br[k, o]
            for i, (o0, n) in enumerate(CHUNKS):
                nc.tensor.matmul(
                    ps[i][:, :], ones[:, :], brt[:, o0 : o0 + n],
                    start=True, stop=False,
                )
            for g in range(NG):
                w = wpool.tile([128, OPC], f16)
                nc.sync.dma_start(w[:], wt[g * 128 : (g + 1) * 128, :])
                for i, (o0, n) in enumerate(CHUNKS):
                    nc.tensor.matmul(
                        ps[i][:, :],
                        xt[:, g * B : (g + 1) * B],
                        w[:, o0 : o0 + n],
                        start=False,
                        stop=(g == NG - 1),
                    )
            osb = opool.tile([B, OPC], f32)
            for i, (o0, n) in enumerate(CHUNKS):
                nc.vector.tensor_copy(osb[:, o0 : o0 + n], ps[i][:, :])
            nc.sync.dma_start(out[:], osb[:])

    nc.compile()
    _NC_CACHE = nc
    return nc


def _prep_inputs(x, weight_q, scale, bias, weight_residual):
    """Host-side shard + layout. Returns in_maps for 8 cores."""
    # x [64, 4096] f32 -> [128 partitions(i within group), 32 groups, 64 batch] f16
    xgh = np.ascontiguousarray(
        x.reshape(B, NG, GS).transpose(2, 1, 0).astype(np.float16)
    ).reshape(128, NG * B)

    in_maps = []
    for c in range(NCORES):
        rows = slice(c * OPC, (c + 1) * OPC)
        wq_c = weight_q[rows]          # [1376, 4096] int8
        sc_c = scale[rows]             # [1376, 32] f16
        # exact fp32 product (int8 * fp16 fits in fp32), single fp16 rounding
        wd = (
            wq_c.reshape(OPC, NG, GS).astype(np.float32)
            * sc_c.astype(np.float32)[:, :, None]
        ).reshape(OPC, IN).astype(np.float16)
        wt_c = np.ascontiguousarray(wd.T)  # [4096, 1376] f16
        br_c = (
            bias[rows].astype(np.float32)
            + weight_residual[rows].astype(np.float32)
        ).astype(np.float16).reshape(1, OPC)
        in_maps.append({"wt": wt_c, "xg": xgh, "br": np.ascontiguousarray(br_c)})
    return in_maps


def kernel(x, weight_q, scale, bias, weight_residual):
    from concourse.bass_utils import run_bass_kernel_spmd

    nc = _build()
    in_maps = _prep_inputs(x, weight_q, scale, bias, weight_residual)
    res = run_bass_kernel_spmd(nc, in_maps, core_ids=list(range(NCORES)))
    out = np.concatenate([res.results[c]["out"] for c in range(NCORES)], axis=1)
    return out.astype(np.float32)
